# revision 17
# baseline (speedup 1.0000x reference)
"""Multi-head causal attention (B=2, L=2048, D=1024, H=16) on 8 trn2 cores.

Sharding: core c -> batch b=c//4, head-group g=c%4 (4 heads / 256 of D).
Each core computes its Q/K/V projections in transposed layout, causal
attention with transposed scores (softmax denominators via a ones-column
appended to V, no max-subtraction needed: |scores/8| <~ 2), and a partial
output projection against its 256-row slice of w_o^T.  The 4 partials per
batch are summed on the host (+ b_o) during unsharding.
"""

import sys

sys.path.insert(0, "/opt/trn_rl_repo")

import numpy as np
import ml_dtypes

import concourse.bass as bass
import concourse.mybir as mybir
import concourse.tile as tile
from concourse.bass_utils import run_bass_kernel_spmd

BF16 = mybir.dt.bfloat16
F32 = mybir.dt.float32
F32R = mybir.dt.float32r

B, L, D, H = 2, 2048, 1024, 16
DK = 64            # head dim
HPC = 4            # heads per core
DS = HPC * DK      # 256: D-slice per core
KT = D // 128      # 8 k-tiles over D
N_CORES = 8
NCH = L // 512     # 4 q-chunks of 512
NSB = 2            # superblocks of 1024 columns


def _split_excess_waits(nc, max_waits=1):
    """The walrus build in this container rejects instructions carrying more
    than `max_waits` sem waits; peel extras onto same-engine NoOps."""
    n_split = 0
    for f in nc.m.functions:
        for bb in f.blocks:
            insts = bb.instructions
            new = []
            changed = False
            for inst in insts:
                si = inst.sync_info
                waits = list(si.on_wait) if si and si.on_wait else []
                if len(waits) > max_waits:
                    changed = True
                    head, keep = waits[:-max_waits], waits[-max_waits:]
                    for i in range(0, len(head), max_waits):
                        nop = mybir.InstNoOp(
                            name=f"wsplit-{inst.name}-{n_split}", ins=[], outs=[])
                        n_split += 1
                        nop.engine = inst.engine
                        nop.sync_info = mybir.SyncInfo(
                            on_wait=head[i:i + max_waits], on_update=[])
                        new.append(nop)
                    inst.sync_info = mybir.SyncInfo(
                        on_wait=keep,
                        on_update=list(si.on_update) if si.on_update else [])
                new.append(inst)
            if changed:
                bb.instructions = new
    return n_split


def _build_nc():
    nc = bass.Bass("TRN2", target_bir_lowering=False, debug=False)

    aps = {}
    for nm, shape, dt in (
        ("xqT", [D, L], BF16), ("xkT", [D, L], BF16), ("xvT", [D, L], BF16),
        ("wqT", [D, DS], BF16), ("wkT", [D, DS], BF16), ("wvT", [D, DS], BF16),
        ("woT", [DS, D], BF16), ("bqk", [128, 4], F32), ("bv", [1, DS], F32),
        ("masks", [4, 128, 512], BF16),
    ):
        aps[nm] = nc.dram_tensor(nm, shape, dt, kind="ExternalInput").ap()
    aps["outT"] = nc.dram_tensor("outT", [D, L], BF16, kind="ExternalOutput").ap()

    with nc.allow_low_precision("bf16 attention intermediates"), \
            tile.TileContext(nc) as tc:
        _emit(nc, tc, aps)

    _split_excess_waits(nc, 1)
    return nc


def _emit(nc, tc, aps):
    mm = nc.tensor.matmul
    ts = bass.ts

    with tc.tile_pool(name="const", bufs=1) as cpool, \
            tc.tile_pool(name="qkv", bufs=1) as qkv:
        # weights: [128, kt, *] so [:, k, m-slice] is a [128, <=512] lhsT
        wq_s = cpool.tile([128, KT, DS], BF16, name="wq_s")
        wk_s = cpool.tile([128, KT, DS], BF16, name="wk_s")
        wv_s = cpool.tile([128, KT, DS], BF16, name="wv_s")
        wo_s = cpool.tile([128, 2, D], BF16, name="wo_s")
        bqk_s = cpool.tile([128, 4], F32, name="bqk_s")
        bv_s = cpool.tile([1, DS], F32, name="bv_s")
        ones_s = cpool.tile([128, 128], F32, name="ones_s")
        ones_r = cpool.tile([128, 64], F32R, name="ones_r")
        mask_s = cpool.tile([128, 4, 512], BF16, name="mask_s")
        bvb_s = cpool.tile([128, DS], BF16, name="bvb_s")

        for t, src in ((wq_s, aps["wqT"]), (wk_s, aps["wkT"]), (wv_s, aps["wvT"])):
            nc.gpsimd.dma_start(out=t[:, :, :],
                                in_=src.rearrange("(k p) m -> p k m", p=128))
        nc.gpsimd.dma_start(out=wo_s[:, :, :],
                            in_=aps["woT"].rearrange("(k p) m -> p k m", p=128))
        nc.gpsimd.dma_start(out=bqk_s[:, :], in_=aps["bqk"])
        nc.gpsimd.dma_start(out=bv_s[:, :], in_=aps["bv"])
        nc.gpsimd.dma_start(out=mask_s[:, :, :],
                            in_=aps["masks"].rearrange("r p j -> p r j"))
        nc.vector.memset(ones_s[:, :], 1.0)
        nc.scalar.copy(ones_r[:, :], ones_s[:, 0:64])

        QT = [qkv.tile([128, L], BF16, name=f"QT{i}") for i in range(2)]
        KTt = [qkv.tile([128, L], BF16, name=f"KTt{i}") for i in range(2)]
        V2 = qkv.tile([128, 16, HPC * 65], BF16, name="V2")
        OTn = [qkv.tile([128, L], BF16, name=f"OTn{i}") for i in range(2)]

        # ones columns of V' (col 64 of each head's 65-wide group): memset the
        # whole tile to 1.0; the V-projection overwrites the 64 value columns.
        nc.vector.memset(V2[:, :, :], 1.0)

        # ---- projections ----
        with tc.tile_pool(name="xs", bufs=1) as xs, \
                tc.tile_pool(name="psA", bufs=1, space="PSUM") as psA:
            # bv broadcast [1,DS] -> [128,DS] via PE ones outer-product
            bvb_ps = psA.tile([128, DS], F32, tag="bvps")
            mm(bvb_ps[:, :], ones_s[0:1, :], bv_s[0:1, :], start=True, stop=True)
            nc.vector.tensor_copy(bvb_s[:, :], bvb_ps[:, :])

            # resident x tiles, one big fully-contiguous DMA each, spread
            # over both HWDGE rings + SWDGE
            xq_t = xs.tile([128, KT, L], BF16, name="xq_t")
            xk_t = xs.tile([128, KT, L], BF16, name="xk_t")
            xv_t = xs.tile([128, KT, L], BF16, name="xv_t")
            nc.sync.dma_start(
                out=xq_t[:, :, :], in_=aps["xqT"].rearrange("(k p) l -> p k l", p=128))
            nc.scalar.dma_start(
                out=xk_t[:, :, :], in_=aps["xkT"].rearrange("(k p) l -> p k l", p=128))
            nc.gpsimd.dma_start(
                out=xv_t[:, :, :], in_=aps["xvT"].rearrange("(k p) l -> p k l", p=128))

            # Q/K: out[d'128, l512] accumulated over 8 k-tiles
            for n in range(NCH):
                for m in range(2):
                    pq = psA.tile([128, 512], F32, tag="qk", bufs=2)
                    for k in range(KT):
                        mm(pq[:, :], wq_s[:, k, ts(m, 128)], xq_t[:, k, ts(n, 512)],
                           start=(k == 0), stop=(k == KT - 1))
                    nc.vector.tensor_scalar_add(
                        QT[m][:, ts(n, 512)], pq[:, :], bqk_s[:, m:m + 1])
                    pk = psA.tile([128, 512], F32, tag="qk", bufs=2)
                    for k in range(KT):
                        mm(pk[:, :], wk_s[:, k, ts(m, 128)], xk_t[:, k, ts(n, 512)],
                           start=(k == 0), stop=(k == KT - 1))
                    nc.vector.tensor_scalar_add(
                        KTt[m][:, ts(n, 512)], pk[:, :], bqk_s[:, m + 2:m + 3])

            # V: out[l128, d'256], natural layout into V2 (+bias broadcast)
            for lt in range(16):
                pvp = psA.tile([128, DS], F32, tag="v", bufs=2)
                for k in range(KT):
                    mm(pvp[:, :], xv_t[:, k, ts(lt, 128)], wv_s[:, k, :],
                       start=(k == 0), stop=(k == KT - 1))
                nc.vector.tensor_tensor(
                    V2[:, lt:lt + 1, :].rearrange("p o (h c) -> p (o h) c", c=65)[:, :, 0:64],
                    pvp[:, :].rearrange("p (h c) -> p h c", c=64),
                    bvb_s[:, :].rearrange("p (h c) -> p h c", c=64),
                    mybir.AluOpType.add)

        # ---- attention, head by head ----
        with tc.tile_pool(name="att", bufs=1) as att, \
                tc.tile_pool(name="psB", bufs=1, space="PSUM") as psB:
            for h in range(HPC):
                po = (h % 2) * 64
                qt = QT[h // 2]
                kt_ = KTt[h // 2]
                for sb in range(NSB):
                    base = sb * 2          # first global 512-chunk of superblock
                    pv = psB.tile([65, 1024], F32, tag="pv", bufs=2)
                    nkl = 8 * (sb + 1)

                    def emit_st(kl):
                        c0 = max(base, kl // 4)
                        loc0 = (c0 - base) * 512
                        st = psB.tile([128, 1024], F32, tag="st", bufs=2,
                                      name="st")
                        est = att.tile([128, 1024], BF16, tag="est", bufs=3,
                                       name="est")
                        for c in range(c0, base + 2):
                            lo = (c - base) * 512
                            mm(st[:, lo:lo + 512],
                               kt_[po:po + 64, ts(kl, 128)],
                               qt[po:po + 64, ts(c, 512)],
                               start=True, stop=True)
                        nc.scalar.activation(
                            est[:, loc0:1024], st[:, loc0:1024],
                            mybir.ActivationFunctionType.Exp, scale=0.125)
                        if kl >= 8 * sb:   # diagonal k-tile: mask partial chunk
                            dloc = (kl // 4 - base) * 512
                            nc.vector.tensor_tensor(
                                est[:, dloc:dloc + 512],
                                est[:, dloc:dloc + 512],
                                mask_s[:, kl % 4, :],
                                mybir.AluOpType.mult)
                        return est

                    def emit_pv(kl, est):
                        c0 = max(base, kl // 4)
                        for c in range(c0, base + 2):
                            lo = (c - base) * 512
                            last_kl = min(nkl - 1, 4 * c + 3)
                            mm(pv[:, lo:lo + 512],
                               V2[:, kl, h * 65:(h + 1) * 65],
                               est[:, lo:lo + 512],
                               start=(kl == 0), stop=(kl == last_kl))

                    # software pipeline: ST(k+1) is emitted before PV(k) so the
                    # in-order PE never stalls on exp(k)
                    prev_est = emit_st(0)
                    for kl in range(1, nkl):
                        est = emit_st(kl)
                        emit_pv(kl - 1, prev_est)
                        prev_est = est
                    emit_pv(nkl - 1, prev_est)
                    # normalize: bcast sums row, reciprocal, multiply
                    sums = att.tile([65, 1024], F32R, tag="sums", bufs=2)
                    nc.scalar.copy(sums[64:65, :], pv[64:65, :])
                    bc = psB.tile([64, 1024], F32, tag="pv", bufs=2)
                    for c in range(2):
                        mm(bc[:, ts(c, 512)],
                           ones_r[64:65, :],
                           sums[64:65, ts(c, 512)],
                           start=True, stop=True)
                    rec = att.tile([64, 1024], F32, tag="rec", bufs=2)
                    nc.vector.reciprocal(rec[:, :], bc[:, :])
                    if po == 0:
                        nc.vector.tensor_tensor(
                            OTn[h // 2][0:64, ts(sb, 1024)],
                            pv[0:64, :], rec[:, :], mybir.AluOpType.mult)
                    else:
                        osc = att.tile([64, 1024], BF16, tag="osc", bufs=2)
                        nc.vector.tensor_tensor(
                            osc[:, :], pv[0:64, :], rec[:, :],
                            mybir.AluOpType.mult)
                        nc.gpsimd.dma_start(
                            out=OTn[h // 2][64:128, ts(sb, 1024)],
                            in_=osc[:, :])

        # ---- final projection: partialT[d_out, l] = woT_g^T . OTn ----
        with tc.tile_pool(name="fin", bufs=1) as fin, \
                tc.tile_pool(name="psC", bufs=1, space="PSUM") as psC:
            for mt in range(8):
                for n in range(NCH):
                    op_ = psC.tile([128, 512], F32, tag="o", bufs=3)
                    mm(op_[:, :], wo_s[:, 0, ts(mt, 128)], OTn[0][:, ts(n, 512)],
                       start=True, stop=False)
                    mm(op_[:, :], wo_s[:, 1, ts(mt, 128)], OTn[1][:, ts(n, 512)],
                       start=False, stop=True)
                    ob = fin.tile([128, 512], BF16, tag="ob", bufs=4)
                    nc.vector.tensor_copy(ob[:, :], op_[:, :])
                    eng = nc.sync if (mt * NCH + n) % 2 == 0 else nc.scalar
                    eng.dma_start(
                        out=aps["outT"][mt * 128:(mt + 1) * 128, ts(n, 512)],
                        in_=ob[:, :])


_NC_CACHE = None


def _get_nc():
    global _NC_CACHE
    if _NC_CACHE is None:
        _NC_CACHE = _build_nc()
    return _NC_CACHE


def _host_prep(inputs):
    bf16 = ml_dtypes.bfloat16
    q = np.asarray(inputs["query"], np.float32)
    k = np.asarray(inputs["key_"], np.float32)
    v = np.asarray(inputs["value"], np.float32)
    w_q = np.asarray(inputs["w_q"], np.float32)
    w_k = np.asarray(inputs["w_k"], np.float32)
    w_v = np.asarray(inputs["w_v"], np.float32)
    w_o = np.asarray(inputs["w_o"], np.float32)
    b_q = np.asarray(inputs["b_q"], np.float32)
    b_k = np.asarray(inputs["b_k"], np.float32)
    b_v = np.asarray(inputs["b_v"], np.float32)

    # causal diagonal-block masks: mask[r][p, j] = (j - 128*r - p) >= 0
    jj = np.arange(512)[None, None, :]
    pp = np.arange(128)[None, :, None]
    rr = np.arange(4)[:, None, None]
    masks = ((jj - 128 * rr - pp) >= 0).astype(bf16)

    xT = {}
    for b in range(B):
        xT[b] = (
            np.ascontiguousarray(q[b].T).astype(bf16),
            np.ascontiguousarray(k[b].T).astype(bf16),
            np.ascontiguousarray(v[b].T).astype(bf16),
        )

    in_maps = []
    for c in range(N_CORES):
        b, g = divmod(c, 4)
        sl = slice(g * DS, (g + 1) * DS)
        bqk = np.stack([
            b_q[sl][0:128], b_q[sl][128:256],
            b_k[sl][0:128], b_k[sl][128:256],
        ], axis=1).astype(np.float32)            # [128, 4]
        in_maps.append({
            "xqT": xT[b][0], "xkT": xT[b][1], "xvT": xT[b][2],
            "wqT": np.ascontiguousarray(w_q[sl, :].T).astype(bf16),
            "wkT": np.ascontiguousarray(w_k[sl, :].T).astype(bf16),
            "wvT": np.ascontiguousarray(w_v[sl, :].T).astype(bf16),
            "woT": np.ascontiguousarray(w_o[:, sl].T).astype(bf16),
            "bqk": bqk,
            "bv": b_v[sl].reshape(1, DS).astype(np.float32),
            "masks": masks,
        })
    return in_maps


def kernel(**inputs):
    nc = _get_nc()
    in_maps = _host_prep(inputs)
    res = run_bass_kernel_spmd(
        nc, in_maps, core_ids=list(range(N_CORES)), trace=False)
    b_o = np.asarray(inputs["b_o"], np.float32)
    out = np.empty((B, L, D), np.float32)
    for b in range(B):
        acc = np.zeros((D, L), np.float32)
        for g in range(4):
            acc += res.results[b * 4 + g]["outT"].astype(np.float32)
        out[b] = acc.T + b_o
    return out


# revision 23
# speedup vs baseline: 1.9699x; 1.9699x over previous
"""Multi-head causal attention (B=2, L=2048, D=1024, H=16) on 8 trn2 cores.

Sharding: core c -> batch b=c//4, head-group g=c%4 (4 heads / 256 of D).
Each core computes its Q/K/V projections in transposed layout, causal
attention with transposed scores (softmax denominators via a ones-column
appended to V, no max-subtraction needed: |scores/8| <~ 2), and a partial
output projection against its 256-row slice of w_o^T.  The 4 partials per
batch are summed on the host (+ b_o) during unsharding.
"""

import sys

sys.path.insert(0, "/opt/trn_rl_repo")

import numpy as np
import ml_dtypes

import concourse.bass as bass
import concourse.mybir as mybir
import concourse.tile as tile
from concourse.bass_utils import run_bass_kernel_spmd

BF16 = mybir.dt.bfloat16
F32 = mybir.dt.float32
F32R = mybir.dt.float32r

B, L, D, H = 2, 2048, 1024, 16
DK = 64            # head dim
HPC = 4            # heads per core
DS = HPC * DK      # 256: D-slice per core
KT = D // 128      # 8 k-tiles over D
N_CORES = 8
NCH = L // 512     # 4 q-chunks of 512
NSB = 2            # superblocks of 1024 columns


def _split_excess_waits(nc, max_waits=1):
    """The walrus build in this container rejects instructions carrying more
    than `max_waits` sem waits; peel extras onto same-engine NoOps."""
    n_split = 0
    for f in nc.m.functions:
        for bb in f.blocks:
            insts = bb.instructions
            new = []
            changed = False
            for inst in insts:
                si = inst.sync_info
                waits = list(si.on_wait) if si and si.on_wait else []
                if len(waits) > max_waits:
                    changed = True
                    head, keep = waits[:-max_waits], waits[-max_waits:]
                    for i in range(0, len(head), max_waits):
                        nop = mybir.InstNoOp(
                            name=f"wsplit-{inst.name}-{n_split}", ins=[], outs=[])
                        n_split += 1
                        nop.engine = inst.engine
                        nop.sync_info = mybir.SyncInfo(
                            on_wait=head[i:i + max_waits], on_update=[])
                        new.append(nop)
                    inst.sync_info = mybir.SyncInfo(
                        on_wait=keep,
                        on_update=list(si.on_update) if si.on_update else [])
                new.append(inst)
            if changed:
                bb.instructions = new
    return n_split


def _build_nc():
    nc = bass.Bass("TRN2", target_bir_lowering=False, debug=False)

    aps = {}
    for nm, shape, dt in (
        ("xqT", [D, L], BF16), ("xkT", [D, L], BF16), ("xvT", [D, L], BF16),
        ("wqT", [D, DS], BF16), ("wkT", [D, DS], BF16), ("wvT", [D, DS], BF16),
        ("woT", [DS, D], BF16), ("bqk", [128, 4], F32), ("bv", [1, DS], F32),
        ("masks", [4, 128, 512], BF16),
    ):
        aps[nm] = nc.dram_tensor(nm, shape, dt, kind="ExternalInput").ap()
    aps["outT"] = nc.dram_tensor("outT", [D, L], BF16, kind="ExternalOutput").ap()

    with nc.allow_low_precision("bf16 attention intermediates"), \
            tile.TileContext(nc) as tc:
        _emit(nc, tc, aps)

    _split_excess_waits(nc, 1)
    return nc


def _emit(nc, tc, aps):
    mm = nc.tensor.matmul
    ts = bass.ts

    with tc.tile_pool(name="const", bufs=1) as cpool, \
            tc.tile_pool(name="qkv", bufs=1) as qkv:
        # weights: [128, kt, *] so [:, k, m-slice] is a [128, <=512] lhsT
        wq_s = cpool.tile([128, KT, DS], BF16, name="wq_s")
        wk_s = cpool.tile([128, KT, DS], BF16, name="wk_s")
        wv_s = cpool.tile([128, KT, DS], BF16, name="wv_s")
        wo_s = cpool.tile([128, 2, D], BF16, name="wo_s")
        bqk_s = cpool.tile([128, 4], F32, name="bqk_s")
        bv_s = cpool.tile([1, DS], F32, name="bv_s")
        ones_s = cpool.tile([128, 128], F32, name="ones_s")
        ones_r = cpool.tile([128, 64], F32R, name="ones_r")
        mask_s = cpool.tile([128, 4, 512], BF16, name="mask_s")
        bvb_s = cpool.tile([128, DS], BF16, name="bvb_s")

        for t, src in ((wq_s, aps["wqT"]), (wk_s, aps["wkT"]), (wv_s, aps["wvT"])):
            nc.gpsimd.dma_start(out=t[:, :, :],
                                in_=src.rearrange("(k p) m -> p k m", p=128))
        nc.gpsimd.dma_start(out=wo_s[:, :, :],
                            in_=aps["woT"].rearrange("(k p) m -> p k m", p=128))
        nc.gpsimd.dma_start(out=bqk_s[:, :], in_=aps["bqk"])
        nc.gpsimd.dma_start(out=bv_s[:, :], in_=aps["bv"])
        nc.gpsimd.dma_start(out=mask_s[:, :, :],
                            in_=aps["masks"].rearrange("r p j -> p r j"))
        nc.vector.memset(ones_s[:, :], 1.0)
        nc.scalar.copy(ones_r[:, :], ones_s[:, 0:64])

        QT = [qkv.tile([128, L], BF16, name=f"QT{i}") for i in range(2)]
        KTt = [qkv.tile([128, L], BF16, name=f"KTt{i}") for i in range(2)]
        V2 = qkv.tile([128, 16, HPC * 65], BF16, name="V2")
        OTn = [qkv.tile([128, L], BF16, name=f"OTn{i}") for i in range(2)]

        # ones columns of V' (col 64 of each head's 65-wide group): memset the
        # whole tile to 1.0; the V-projection overwrites the 64 value columns.
        nc.vector.memset(V2[:, :, :], 1.0)

        # ---- projections ----
        with tc.tile_pool(name="xs", bufs=1) as xs, \
                tc.tile_pool(name="psA", bufs=1, space="PSUM") as psA:
            # bv broadcast [1,DS] -> [128,DS] via PE ones outer-product
            bvb_ps = psA.tile([128, DS], F32, tag="bvps")
            mm(bvb_ps[:, :], ones_s[0:1, :], bv_s[0:1, :], start=True, stop=True)
            nc.vector.tensor_copy(bvb_s[:, :], bvb_ps[:, :])

            # resident x tiles, one big fully-contiguous DMA each, spread
            # over both HWDGE rings + SWDGE
            xq_t = xs.tile([128, KT, L], BF16, name="xq_t")
            xk_t = xs.tile([128, KT, L], BF16, name="xk_t")
            xv_t = xs.tile([128, KT, L], BF16, name="xv_t")
            for k in range(KT):
                nc.sync.dma_start(
                    out=xq_t[:, k, :],
                    in_=aps["xqT"].rearrange("(k p) l -> p k l", p=128)[:, k, :])
                nc.scalar.dma_start(
                    out=xk_t[:, k, :],
                    in_=aps["xkT"].rearrange("(k p) l -> p k l", p=128)[:, k, :])
                nc.gpsimd.dma_start(
                    out=xv_t[:, k, :],
                    in_=aps["xvT"].rearrange("(k p) l -> p k l", p=128)[:, k, :])

            # Q/K: out[d'128, l512] accumulated over 8 k-tiles; m-outer so the
            # head pair 0/1 tiles (m=0) complete first and attention can start
            for m in range(2):
                for n in range(NCH):
                    pq = psA.tile([128, 512], F32, tag="qk", bufs=3)
                    for k in range(KT):
                        mm(pq[:, :], wq_s[:, k, ts(m, 128)], xq_t[:, k, ts(n, 512)],
                           start=(k == 0), stop=(k == KT - 1))
                    nc.vector.tensor_scalar_add(
                        QT[m][:, ts(n, 512)], pq[:, :], bqk_s[:, m:m + 1])
                    pk = psA.tile([128, 512], F32, tag="qk", bufs=3)
                    for k in range(KT):
                        mm(pk[:, :], wk_s[:, k, ts(m, 128)], xk_t[:, k, ts(n, 512)],
                           start=(k == 0), stop=(k == KT - 1))
                    nc.vector.tensor_scalar_add(
                        KTt[m][:, ts(n, 512)], pk[:, :], bqk_s[:, m + 2:m + 3])

            # V: out[l128, d'256], natural layout into V2 (+bias broadcast)
            for lt in range(16):
                pvp = psA.tile([128, DS], F32, tag="v", bufs=3)
                for k in range(KT):
                    mm(pvp[:, :], xv_t[:, k, ts(lt, 128)], wv_s[:, k, :],
                       start=(k == 0), stop=(k == KT - 1))
                nc.vector.tensor_tensor(
                    V2[:, lt:lt + 1, :].rearrange("p o (h c) -> p (o h) c", c=65)[:, :, 0:64],
                    pvp[:, :].rearrange("p (h c) -> p h c", c=64),
                    bvb_s[:, :].rearrange("p (h c) -> p h c", c=64),
                    mybir.AluOpType.add)

        # ---- attention, head by head ----
        with tc.tile_pool(name="att", bufs=1) as att, \
                tc.tile_pool(name="psB", bufs=1, space="PSUM") as psB:
            # deferred normalize: emitted after the NEXT block's first STs so
            # the in-order PE never waits on ACT/DVE
            pending = []

            def flush_pending():
                while pending:
                    pending.pop(0)()

            for h in range(HPC):
                po = (h % 2) * 64
                qt = QT[h // 2]
                kt_ = KTt[h // 2]
                for sb in range(NSB):
                    base = sb * 2          # first global 512-chunk of superblock
                    pv = psB.tile([65, 1024], F32, tag="pv", bufs=2)
                    nkl = 8 * (sb + 1)

                    sb0 = base * 512       # global q origin of superblock

                    def emit_st(kl):
                        # causal trim at 128 granularity: only q >= kl*128
                        loc0 = max(0, kl * 128 - sb0)
                        st = psB.tile([128, 1024], F32, tag="st", bufs=2,
                                      name="st")
                        est = att.tile([128, 1024], BF16, tag="est", bufs=3,
                                       name="est")
                        for c in range(2):
                            lo, hi = c * 512, (c + 1) * 512
                            lo = max(lo, loc0)
                            if lo >= hi:
                                continue
                            mm(st[:, lo:hi],
                               kt_[po:po + 64, ts(kl, 128)],
                               qt[po:po + 64, sb0 + lo:sb0 + hi],
                               start=True, stop=True)
                        nc.scalar.activation(
                            est[:, loc0:1024], st[:, loc0:1024],
                            mybir.ActivationFunctionType.Exp, scale=0.125)
                        if kl >= 8 * sb:   # diagonal k-tile: mask first 128 cols
                            nc.vector.tensor_tensor(
                                est[:, loc0:loc0 + 128],
                                est[:, loc0:loc0 + 128],
                                mask_s[:, 0, 0:128],
                                mybir.AluOpType.mult)
                        return est

                    def emit_pv(kl, est):
                        loc0 = max(0, kl * 128 - sb0)
                        for c in range(2):
                            lo, hi = c * 512, (c + 1) * 512
                            lo = max(lo, loc0)
                            if lo >= hi:
                                continue
                            mm(pv[:, lo:hi],
                               V2[:, kl, h * 65:(h + 1) * 65],
                               est[:, lo:hi],
                               start=(kl == 0), stop=(kl == nkl - 1),
                               skip_group_check=True)

                    # software pipeline: ST(k+1) is emitted before PV(k) so the
                    # in-order PE never stalls on exp(k); the previous block's
                    # normalize lands between our first STs
                    prev_est = emit_st(0)
                    first = True
                    for kl in range(1, nkl):
                        est = emit_st(kl)
                        if first:
                            flush_pending()
                            first = False
                        emit_pv(kl - 1, prev_est)
                        prev_est = est
                    emit_pv(nkl - 1, prev_est)
                    # sums row copy now (ACT, runs while next block's STs issue)
                    sums = att.tile([65, 1024], F32R, tag="sums", bufs=2)
                    nc.scalar.copy(sums[64:65, :], pv[64:65, :])

                    def normalize(h=h, sb=sb, pv=pv, sums=sums, po=po):
                        bc = psB.tile([64, 1024], F32, tag="st", bufs=2,
                                      name="bc")
                        for c in range(2):
                            mm(bc[:, ts(c, 512)],
                               ones_r[64:65, :],
                               sums[64:65, ts(c, 512)],
                               start=True, stop=True)
                        rec = att.tile([64, 1024], F32, tag="rec", bufs=2,
                                       name="rec")
                        nc.vector.reciprocal(rec[:, :], bc[:, :])
                        if po == 0:
                            nc.vector.tensor_tensor(
                                OTn[h // 2][0:64, ts(sb, 1024)],
                                pv[0:64, :], rec[:, :], mybir.AluOpType.mult)
                        else:
                            osc = att.tile([64, 1024], BF16, tag="osc", bufs=2,
                                           name="osc")
                            nc.vector.tensor_tensor(
                                osc[:, :], pv[0:64, :], rec[:, :],
                                mybir.AluOpType.mult)
                            nc.gpsimd.dma_start(
                                out=OTn[h // 2][64:128, ts(sb, 1024)],
                                in_=osc[:, :])

                    pending.append(normalize)
            flush_pending()

        # ---- final projection: partialT[d_out, l] = woT_g^T . OTn ----
        with tc.tile_pool(name="fin", bufs=1) as fin, \
                tc.tile_pool(name="psC", bufs=1, space="PSUM") as psC:
            for mt in range(8):
                for n in range(NCH):
                    op_ = psC.tile([128, 512], F32, tag="o", bufs=3)
                    mm(op_[:, :], wo_s[:, 0, ts(mt, 128)], OTn[0][:, ts(n, 512)],
                       start=True, stop=False)
                    mm(op_[:, :], wo_s[:, 1, ts(mt, 128)], OTn[1][:, ts(n, 512)],
                       start=False, stop=True)
                    ob = fin.tile([128, 512], BF16, tag="ob", bufs=4)
                    nc.vector.tensor_copy(ob[:, :], op_[:, :])
                    eng = nc.sync if (mt * NCH + n) % 2 == 0 else nc.scalar
                    eng.dma_start(
                        out=aps["outT"][mt * 128:(mt + 1) * 128, ts(n, 512)],
                        in_=ob[:, :])


_NC_CACHE = None


def _get_nc():
    global _NC_CACHE
    if _NC_CACHE is None:
        _NC_CACHE = _build_nc()
    return _NC_CACHE


def _host_prep(inputs):
    bf16 = ml_dtypes.bfloat16
    q = np.asarray(inputs["query"], np.float32)
    k = np.asarray(inputs["key_"], np.float32)
    v = np.asarray(inputs["value"], np.float32)
    w_q = np.asarray(inputs["w_q"], np.float32)
    w_k = np.asarray(inputs["w_k"], np.float32)
    w_v = np.asarray(inputs["w_v"], np.float32)
    w_o = np.asarray(inputs["w_o"], np.float32)
    b_q = np.asarray(inputs["b_q"], np.float32)
    b_k = np.asarray(inputs["b_k"], np.float32)
    b_v = np.asarray(inputs["b_v"], np.float32)

    # causal diagonal-block masks: mask[r][p, j] = (j - 128*r - p) >= 0
    jj = np.arange(512)[None, None, :]
    pp = np.arange(128)[None, :, None]
    rr = np.arange(4)[:, None, None]
    masks = ((jj - 128 * rr - pp) >= 0).astype(bf16)

    xT = {}
    for b in range(B):
        xT[b] = (
            np.ascontiguousarray(q[b].T).astype(bf16),
            np.ascontiguousarray(k[b].T).astype(bf16),
            np.ascontiguousarray(v[b].T).astype(bf16),
        )

    in_maps = []
    for c in range(N_CORES):
        b, g = divmod(c, 4)
        sl = slice(g * DS, (g + 1) * DS)
        bqk = np.stack([
            b_q[sl][0:128], b_q[sl][128:256],
            b_k[sl][0:128], b_k[sl][128:256],
        ], axis=1).astype(np.float32)            # [128, 4]
        in_maps.append({
            "xqT": xT[b][0], "xkT": xT[b][1], "xvT": xT[b][2],
            "wqT": np.ascontiguousarray(w_q[sl, :].T).astype(bf16),
            "wkT": np.ascontiguousarray(w_k[sl, :].T).astype(bf16),
            "wvT": np.ascontiguousarray(w_v[sl, :].T).astype(bf16),
            "woT": np.ascontiguousarray(w_o[:, sl].T).astype(bf16),
            "bqk": bqk,
            "bv": b_v[sl].reshape(1, DS).astype(np.float32),
            "masks": masks,
        })
    return in_maps


def kernel(**inputs):
    nc = _get_nc()
    in_maps = _host_prep(inputs)
    res = run_bass_kernel_spmd(
        nc, in_maps, core_ids=list(range(N_CORES)), trace=False)
    b_o = np.asarray(inputs["b_o"], np.float32)
    out = np.empty((B, L, D), np.float32)
    for b in range(B):
        acc = np.zeros((D, L), np.float32)
        for g in range(4):
            acc += res.results[b * 4 + g]["outT"].astype(np.float32)
        out[b] = acc.T + b_o
    return out


# revision 24
# speedup vs baseline: 2.0943x; 1.0631x over previous
"""Multi-head causal attention (B=2, L=2048, D=1024, H=16) on 8 trn2 cores.

Sharding: core c -> batch b=c//4, head-group g=c%4 (4 heads / 256 of D).
Each core computes its Q/K/V projections in transposed layout, causal
attention with transposed scores (softmax denominators via a ones-column
appended to V, no max-subtraction needed: |scores/8| <~ 2), and a partial
output projection against its 256-row slice of w_o^T.  The 4 partials per
batch are summed on the host (+ b_o) during unsharding.
"""

import sys

sys.path.insert(0, "/opt/trn_rl_repo")

import numpy as np
import ml_dtypes

import concourse.bass as bass
import concourse.mybir as mybir
import concourse.tile as tile
from concourse.bass_utils import run_bass_kernel_spmd

BF16 = mybir.dt.bfloat16
F32 = mybir.dt.float32
F32R = mybir.dt.float32r

B, L, D, H = 2, 2048, 1024, 16
DK = 64            # head dim
HPC = 4            # heads per core
DS = HPC * DK      # 256: D-slice per core
KT = D // 128      # 8 k-tiles over D
N_CORES = 8
NCH = L // 512     # 4 q-chunks of 512
NSB = 2            # superblocks of 1024 columns


def _split_excess_waits(nc, max_waits=1):
    """The walrus build in this container rejects instructions carrying more
    than `max_waits` sem waits; peel extras onto same-engine NoOps."""
    n_split = 0
    for f in nc.m.functions:
        for bb in f.blocks:
            insts = bb.instructions
            new = []
            changed = False
            for inst in insts:
                si = inst.sync_info
                waits = list(si.on_wait) if si and si.on_wait else []
                if len(waits) > max_waits:
                    changed = True
                    head, keep = waits[:-max_waits], waits[-max_waits:]
                    for i in range(0, len(head), max_waits):
                        nop = mybir.InstNoOp(
                            name=f"wsplit-{inst.name}-{n_split}", ins=[], outs=[])
                        n_split += 1
                        nop.engine = inst.engine
                        nop.sync_info = mybir.SyncInfo(
                            on_wait=head[i:i + max_waits], on_update=[])
                        new.append(nop)
                    inst.sync_info = mybir.SyncInfo(
                        on_wait=keep,
                        on_update=list(si.on_update) if si.on_update else [])
                new.append(inst)
            if changed:
                bb.instructions = new
    return n_split


def _build_nc():
    nc = bass.Bass("TRN2", target_bir_lowering=False, debug=False)

    aps = {}
    for nm, shape, dt in (
        ("xqT", [D, L], BF16), ("xkT", [D, L], BF16), ("xvT", [D, L], BF16),
        ("wqT", [D, DS], BF16), ("wkT", [D, DS], BF16), ("wvT", [D, DS], BF16),
        ("woT", [DS, D], BF16), ("bqk", [128, 4], F32), ("bv", [1, DS], F32),
        ("masks", [4, 128, 512], BF16),
    ):
        aps[nm] = nc.dram_tensor(nm, shape, dt, kind="ExternalInput").ap()
    aps["outT"] = nc.dram_tensor("outT", [D, L], BF16, kind="ExternalOutput").ap()

    with nc.allow_low_precision("bf16 attention intermediates"), \
            tile.TileContext(nc) as tc:
        _emit(nc, tc, aps)

    _split_excess_waits(nc, 1)
    return nc


def _emit(nc, tc, aps):
    mm = nc.tensor.matmul
    ts = bass.ts

    with tc.tile_pool(name="const", bufs=1) as cpool, \
            tc.tile_pool(name="qkv", bufs=1) as qkv:
        # weights: [128, kt, *] so [:, k, m-slice] is a [128, <=512] lhsT
        wq_s = cpool.tile([128, KT, DS], BF16, name="wq_s")
        wk_s = cpool.tile([128, KT, DS], BF16, name="wk_s")
        wv_s = cpool.tile([128, KT, DS], BF16, name="wv_s")
        wo_s = cpool.tile([128, 2, D], BF16, name="wo_s")
        bqk_s = cpool.tile([128, 4], F32, name="bqk_s")
        bv_s = cpool.tile([1, DS], F32, name="bv_s")
        ones_s = cpool.tile([128, 128], F32, name="ones_s")
        ones_r = cpool.tile([128, 64], F32R, name="ones_r")
        mask_s = cpool.tile([128, 4, 512], BF16, name="mask_s")
        bvb_s = cpool.tile([128, DS], BF16, name="bvb_s")

        for t, src in ((wq_s, aps["wqT"]), (wk_s, aps["wkT"]), (wv_s, aps["wvT"])):
            nc.gpsimd.dma_start(out=t[:, :, :],
                                in_=src.rearrange("(k p) m -> p k m", p=128))
        nc.gpsimd.dma_start(out=wo_s[:, :, :],
                            in_=aps["woT"].rearrange("(k p) m -> p k m", p=128))
        nc.gpsimd.dma_start(out=bqk_s[:, :], in_=aps["bqk"])
        nc.gpsimd.dma_start(out=bv_s[:, :], in_=aps["bv"])
        nc.gpsimd.dma_start(out=mask_s[:, :, :],
                            in_=aps["masks"].rearrange("r p j -> p r j"))
        nc.vector.memset(ones_s[:, :], 1.0)
        nc.scalar.copy(ones_r[:, :], ones_s[:, 0:64])

        QT = [qkv.tile([128, L], BF16, name=f"QT{i}") for i in range(2)]
        KTt = [qkv.tile([128, L], BF16, name=f"KTt{i}") for i in range(2)]
        V2 = qkv.tile([128, 16, HPC * 65], BF16, name="V2")
        OTn = [qkv.tile([128, L], BF16, name=f"OTn{i}") for i in range(2)]

        # ones columns of V' (col 64 of each head's 65-wide group): memset the
        # whole tile to 1.0; the V-projection overwrites the 64 value columns.
        nc.vector.memset(V2[:, :, :], 1.0)

        # ---- projections ----
        with tc.tile_pool(name="xs", bufs=1) as xs, \
                tc.tile_pool(name="psA", bufs=1, space="PSUM") as psA:
            # bv broadcast [1,DS] -> [128,DS] via PE ones outer-product
            bvb_ps = psA.tile([128, DS], F32, tag="bvps")
            mm(bvb_ps[:, :], ones_s[0:1, :], bv_s[0:1, :], start=True, stop=True)
            nc.vector.tensor_copy(bvb_s[:, :], bvb_ps[:, :])

            # resident x tiles, one big fully-contiguous DMA each, spread
            # over both HWDGE rings + SWDGE
            xq_t = xs.tile([128, KT, L], BF16, name="xq_t")
            xk_t = xs.tile([128, KT, L], BF16, name="xk_t")
            xv_t = xs.tile([128, KT, L], BF16, name="xv_t")
            for k in range(KT):
                nc.sync.dma_start(
                    out=xq_t[:, k, :],
                    in_=aps["xqT"].rearrange("(k p) l -> p k l", p=128)[:, k, :])
                nc.scalar.dma_start(
                    out=xk_t[:, k, :],
                    in_=aps["xkT"].rearrange("(k p) l -> p k l", p=128)[:, k, :])
                nc.gpsimd.dma_start(
                    out=xv_t[:, k, :],
                    in_=aps["xvT"].rearrange("(k p) l -> p k l", p=128)[:, k, :])

            # Q/K: out[d'128, l512]; k-OUTER accumulation so the PE consumes
            # x k-tiles as the DMAs deliver them (4 psum tiles live per pass)
            for m in range(2):
                for which in range(2):   # 0 = Q, 1 = K
                    w_t = wq_s if which == 0 else wk_s
                    x_t = xq_t if which == 0 else xk_t
                    dst = QT[m] if which == 0 else KTt[m]
                    ps = [psA.tile([128, 512], F32, tag="qk", bufs=4,
                                   name=f"p{which}{m}{n}") for n in range(NCH)]
                    for k in range(KT):
                        for n in range(NCH):
                            mm(ps[n][:, :], w_t[:, k, ts(m, 128)],
                               x_t[:, k, ts(n, 512)],
                               start=(k == 0), stop=(k == KT - 1))
                    for n in range(NCH):
                        nc.vector.tensor_scalar_add(
                            dst[:, ts(n, 512)], ps[n][:, :],
                            bqk_s[:, 2 * which + m:2 * which + m + 1])

            # V: out[l128, d'256], natural layout into V2 (+bias broadcast)
            for lt in range(16):
                pvp = psA.tile([128, DS], F32, tag="v", bufs=3)
                for k in range(KT):
                    mm(pvp[:, :], xv_t[:, k, ts(lt, 128)], wv_s[:, k, :],
                       start=(k == 0), stop=(k == KT - 1))
                nc.vector.tensor_tensor(
                    V2[:, lt:lt + 1, :].rearrange("p o (h c) -> p (o h) c", c=65)[:, :, 0:64],
                    pvp[:, :].rearrange("p (h c) -> p h c", c=64),
                    bvb_s[:, :].rearrange("p (h c) -> p h c", c=64),
                    mybir.AluOpType.add)

        # ---- attention, head by head ----
        with tc.tile_pool(name="att", bufs=1) as att, \
                tc.tile_pool(name="psB", bufs=1, space="PSUM") as psB:
            # deferred normalize: emitted after the NEXT block's first STs so
            # the in-order PE never waits on ACT/DVE
            pending = []

            def flush_pending():
                while pending:
                    pending.pop(0)()

            for h in range(HPC):
                po = (h % 2) * 64
                qt = QT[h // 2]
                kt_ = KTt[h // 2]
                for sb in range(NSB):
                    base = sb * 2          # first global 512-chunk of superblock
                    pv = psB.tile([65, 1024], F32, tag="pv", bufs=2)
                    nkl = 8 * (sb + 1)

                    sb0 = base * 512       # global q origin of superblock

                    def emit_st(kl):
                        # causal trim at 128 granularity: only q >= kl*128
                        loc0 = max(0, kl * 128 - sb0)
                        st = psB.tile([128, 1024], F32, tag="st", bufs=2,
                                      name="st")
                        est = att.tile([128, 1024], BF16, tag="est", bufs=3,
                                       name="est")
                        for c in range(2):
                            lo, hi = c * 512, (c + 1) * 512
                            lo = max(lo, loc0)
                            if lo >= hi:
                                continue
                            mm(st[:, lo:hi],
                               kt_[po:po + 64, ts(kl, 128)],
                               qt[po:po + 64, sb0 + lo:sb0 + hi],
                               start=True, stop=True)
                        nc.scalar.activation(
                            est[:, loc0:1024], st[:, loc0:1024],
                            mybir.ActivationFunctionType.Exp, scale=0.125)
                        if kl >= 8 * sb:   # diagonal k-tile: mask first 128 cols
                            nc.vector.tensor_tensor(
                                est[:, loc0:loc0 + 128],
                                est[:, loc0:loc0 + 128],
                                mask_s[:, 0, 0:128],
                                mybir.AluOpType.mult)
                        return est

                    def emit_pv(kl, est):
                        loc0 = max(0, kl * 128 - sb0)
                        for c in range(2):
                            lo, hi = c * 512, (c + 1) * 512
                            lo = max(lo, loc0)
                            if lo >= hi:
                                continue
                            mm(pv[:, lo:hi],
                               V2[:, kl, h * 65:(h + 1) * 65],
                               est[:, lo:hi],
                               start=(kl == 0), stop=(kl == nkl - 1),
                               skip_group_check=True)

                    # software pipeline: ST(k+1) is emitted before PV(k) so the
                    # in-order PE never stalls on exp(k); the previous block's
                    # normalize lands between our first STs
                    prev_est = emit_st(0)
                    first = True
                    for kl in range(1, nkl):
                        est = emit_st(kl)
                        if first:
                            flush_pending()
                            first = False
                        emit_pv(kl - 1, prev_est)
                        prev_est = est
                    emit_pv(nkl - 1, prev_est)
                    # sums row copy now (ACT, runs while next block's STs issue)
                    sums = att.tile([65, 1024], F32R, tag="sums", bufs=2)
                    nc.scalar.copy(sums[64:65, :], pv[64:65, :])

                    def normalize(h=h, sb=sb, pv=pv, sums=sums, po=po):
                        bc = psB.tile([64, 1024], F32, tag="st", bufs=2,
                                      name="bc")
                        for c in range(2):
                            mm(bc[:, ts(c, 512)],
                               ones_r[64:65, :],
                               sums[64:65, ts(c, 512)],
                               start=True, stop=True)
                        rec = att.tile([64, 1024], F32, tag="rec", bufs=2,
                                       name="rec")
                        nc.vector.reciprocal(rec[:, :], bc[:, :])
                        if po == 0:
                            nc.vector.tensor_tensor(
                                OTn[h // 2][0:64, ts(sb, 1024)],
                                pv[0:64, :], rec[:, :], mybir.AluOpType.mult)
                        else:
                            osc = att.tile([64, 1024], BF16, tag="osc", bufs=2,
                                           name="osc")
                            nc.vector.tensor_tensor(
                                osc[:, :], pv[0:64, :], rec[:, :],
                                mybir.AluOpType.mult)
                            nc.gpsimd.dma_start(
                                out=OTn[h // 2][64:128, ts(sb, 1024)],
                                in_=osc[:, :])

                    pending.append(normalize)
            flush_pending()

        # ---- final projection: partialT[d_out, l] = woT_g^T . OTn ----
        with tc.tile_pool(name="fin", bufs=1) as fin, \
                tc.tile_pool(name="psC", bufs=1, space="PSUM") as psC:
            for mt in range(8):
                for n in range(NCH):
                    op_ = psC.tile([128, 512], F32, tag="o", bufs=3)
                    mm(op_[:, :], wo_s[:, 0, ts(mt, 128)], OTn[0][:, ts(n, 512)],
                       start=True, stop=False)
                    mm(op_[:, :], wo_s[:, 1, ts(mt, 128)], OTn[1][:, ts(n, 512)],
                       start=False, stop=True)
                    ob = fin.tile([128, 512], BF16, tag="ob", bufs=4)
                    nc.vector.tensor_copy(ob[:, :], op_[:, :])
                    eng = nc.sync if (mt * NCH + n) % 2 == 0 else nc.scalar
                    eng.dma_start(
                        out=aps["outT"][mt * 128:(mt + 1) * 128, ts(n, 512)],
                        in_=ob[:, :])


_NC_CACHE = None


def _get_nc():
    global _NC_CACHE
    if _NC_CACHE is None:
        _NC_CACHE = _build_nc()
    return _NC_CACHE


def _host_prep(inputs):
    bf16 = ml_dtypes.bfloat16
    q = np.asarray(inputs["query"], np.float32)
    k = np.asarray(inputs["key_"], np.float32)
    v = np.asarray(inputs["value"], np.float32)
    w_q = np.asarray(inputs["w_q"], np.float32)
    w_k = np.asarray(inputs["w_k"], np.float32)
    w_v = np.asarray(inputs["w_v"], np.float32)
    w_o = np.asarray(inputs["w_o"], np.float32)
    b_q = np.asarray(inputs["b_q"], np.float32)
    b_k = np.asarray(inputs["b_k"], np.float32)
    b_v = np.asarray(inputs["b_v"], np.float32)

    # causal diagonal-block masks: mask[r][p, j] = (j - 128*r - p) >= 0
    jj = np.arange(512)[None, None, :]
    pp = np.arange(128)[None, :, None]
    rr = np.arange(4)[:, None, None]
    masks = ((jj - 128 * rr - pp) >= 0).astype(bf16)

    xT = {}
    for b in range(B):
        xT[b] = (
            np.ascontiguousarray(q[b].T).astype(bf16),
            np.ascontiguousarray(k[b].T).astype(bf16),
            np.ascontiguousarray(v[b].T).astype(bf16),
        )

    in_maps = []
    for c in range(N_CORES):
        b, g = divmod(c, 4)
        sl = slice(g * DS, (g + 1) * DS)
        bqk = np.stack([
            b_q[sl][0:128], b_q[sl][128:256],
            b_k[sl][0:128], b_k[sl][128:256],
        ], axis=1).astype(np.float32)            # [128, 4]
        in_maps.append({
            "xqT": xT[b][0], "xkT": xT[b][1], "xvT": xT[b][2],
            "wqT": np.ascontiguousarray(w_q[sl, :].T).astype(bf16),
            "wkT": np.ascontiguousarray(w_k[sl, :].T).astype(bf16),
            "wvT": np.ascontiguousarray(w_v[sl, :].T).astype(bf16),
            "woT": np.ascontiguousarray(w_o[:, sl].T).astype(bf16),
            "bqk": bqk,
            "bv": b_v[sl].reshape(1, DS).astype(np.float32),
            "masks": masks,
        })
    return in_maps


def kernel(**inputs):
    nc = _get_nc()
    in_maps = _host_prep(inputs)
    res = run_bass_kernel_spmd(
        nc, in_maps, core_ids=list(range(N_CORES)), trace=False)
    b_o = np.asarray(inputs["b_o"], np.float32)
    out = np.empty((B, L, D), np.float32)
    for b in range(B):
        acc = np.zeros((D, L), np.float32)
        for g in range(4):
            acc += res.results[b * 4 + g]["outT"].astype(np.float32)
        out[b] = acc.T + b_o
    return out
